# revision 4
# baseline (speedup 1.0000x reference)
"""Trainium2 Bass kernel for nn_CausalMultiTokenPredictionHead.

Distribution: pure data parallel over the flattened B*T axis (1024 sequences
-> 128 per core x 8 cores). Each core runs the full 3-token causal decoder
layer for its 128 sequences and projects its 384 tokens against the full
(padded) vocab. Decoder weights + the vocab projection table are replicated.

Math notes (exact simplifications, no approximations beyond bf16 rounding):
  - Cross-attention has memory length 1 -> softmax over a single key is
    identically 1, so ca(x) = out_proj(v_proj(mem)) independent of x. The two
    projections collapse into ONE host-folded [H,H] matrix; the ln1 beta is
    folded into its bias.
  - Self-attention is over 3 tokens with a causal mask -> per-position
    closed-form softmax over <=3 scores on the vector engine. Scores are
    bounded (|s| < ~3) so the max-subtraction is skipped; 1/sqrt(hd) is
    folded into Wq on the host.
  - Position 0 attends only to itself, so o(p0) = v0; the kernel pushes
    position 0 through the decoder first so the vocab projection can start
    ~25us in and fill the tensor engine while positions 1/2 finish. The
    first few vocab weight groups are re-streamed later for p1/p2.
  - qkv is packed [v|k|q] so position 0 only computes its v columns.
  - LN2's affine folds into lin1_w/lin1_b (residual side applies g2 on DVE);
    LN3's affine folds into proj_w, with the beta3 @ proj_w^T vector added
    on the host after download (it is all-zeros in practice).
  - The tgt residual + sa_out bias are folded into a token-major upload of
    the target embeddings.
All matmuls run in bf16 (fp32 PSUM accumulation); layernorms, softmax and
the residual stream are fp32. All PE transposes run in bf16 (1 cycle/row).
Logits are written to DRAM as bf16 and upcast on the host.
"""
import numpy as np
import ml_dtypes

import concourse.bass as bass
import concourse.mybir as mybir
import concourse.tile as tile
from concourse import bacc
from concourse.bass_utils import run_bass_kernel_spmd
from concourse.masks import make_identity

BF16 = ml_dtypes.bfloat16
F32 = mybir.dt.float32
BF = mybir.dt.bfloat16

B, T, H, V, NT, NH, DFF = 2, 512, 768, 51865, 3, 4, 2048
EPS = 1e-5
NCORES = 8
S = 128                       # sequences per core
TOK = S * NT                  # tokens per core (pos-major: t = p*128 + s)
HT = H // 128                 # 6 h-tiles
FT = DFF // 128               # 16 dff-tiles
HD = H // NH                  # 192 head dim
VP = 52224                    # padded vocab (102 * 512)
VG = 1024                     # vocab columns per streamed weight group
NVG = VP // VG                # 51 groups
ACT = mybir.ActivationFunctionType
ALU = mybir.AluOpType

# packed qkv column chunks (<=512 for one PSUM bank); layout [v|k|q]
CH = [(0, 512), (512, 512), (1024, 512), (1536, 512), (2048, 256)]
CH0 = [(0, 512), (512, 256)]   # p0 only needs v (cols 0:768)
CH_H = [(0, 512), (512, 256)]  # 768 split

# projection job schedule: how many leading vocab groups run before
# x3T[p1] / x3T[p2] are ready (those groups are re-streamed later).
K0 = 3   # groups done p0-only
K1 = 6   # groups done p0+p1


def _bcast_load(nc, pool, dram, n, name, dtype=BF):
    """[n] DRAM vector -> [128, n] SBUF tile broadcast across partitions."""
    t = pool.tile([128, n], dtype, name=name, tag=name)
    ap = dram[:]
    bc = bass.AP(tensor=ap.tensor, offset=ap.offset, ap=[[0, 128]] + list(ap.ap))
    nc.gpsimd.dma_start(out=t[:], in_=bc)
    return t


def build_program():
    nc = bacc.Bacc(None, target_bir_lowering=False)

    # ---- DRAM I/O ----
    xT_d = nc.dram_tensor("xT", [H, TOK], BF, kind="ExternalInput")
    xtok_d = nc.dram_tensor("xtok", [S, NT, H], BF, kind="ExternalInput")
    memT_d = nc.dram_tensor("memT", [H, S], BF, kind="ExternalInput")
    wqkvT_d = nc.dram_tensor("wqkvT", [H, 3 * H], BF, kind="ExternalInput")
    woT_d = nc.dram_tensor("woT", [H, H], BF, kind="ExternalInput")
    wcaT_d = nc.dram_tensor("wcaT", [H, H], BF, kind="ExternalInput")
    w1T_d = nc.dram_tensor("w1T", [H, DFF], BF, kind="ExternalInput")
    w2T_d = nc.dram_tensor("w2T", [DFF, H], BF, kind="ExternalInput")
    projT_d = nc.dram_tensor("projT", [H, VP], BF, kind="ExternalInput")
    bqkv_d = nc.dram_tensor("bqkv", [3 * H], BF, kind="ExternalInput")
    bca_d = nc.dram_tensor("bca", [H], BF, kind="ExternalInput")
    bres_d = nc.dram_tensor("bres", [H], BF, kind="ExternalInput")
    b1_d = nc.dram_tensor("b1", [DFF], F32, kind="ExternalInput")
    g1_d = nc.dram_tensor("g1", [H], BF, kind="ExternalInput")
    g2_d = nc.dram_tensor("g2", [H], BF, kind="ExternalInput")
    out_d = nc.dram_tensor("out", [S, NT, VP], BF, kind="ExternalOutput")

    with tile.TileContext(nc) as tc:
        consts = tc.alloc_tile_pool(name="consts", bufs=1)
        longl = tc.alloc_tile_pool(name="longl", bufs=1)
        projp = tc.alloc_tile_pool(name="projp", bufs=3)
        stagep = tc.alloc_tile_pool(name="stagep", bufs=3)
        tmpp = tc.alloc_tile_pool(name="tmpp", bufs=1)
        wbig = tc.alloc_tile_pool(name="wbig", bufs=2)
        ffnp = tc.alloc_tile_pool(name="ffnp", bufs=1)
        psmm = tc.alloc_tile_pool(name="psmm", bufs=6, space="PSUM")
        pstp = tc.alloc_tile_pool(name="pstp", bufs=2, space="PSUM")

        # ---- constants ----
        ident_bf = consts.tile([128, 128], BF, name="ident_bf", tag="ident_bf")
        make_identity(nc, ident_bf)
        epst = consts.tile([128, 1], F32, name="epst", tag="epst")
        nc.vector.memset(epst, EPS)

        # ---- long-lived activations ----
        x3T = longl.tile([128, HT, TOK], BF, name="x3T", tag="x3T")
        x2T = longl.tile([128, HT, TOK], BF, name="x2T", tag="x2T")
        h1p_t = {}

        def scratch(name):
            return tmpp.tile([128, H], F32, name=name, tag="scratch", bufs=3)

        def ln_inplace(x_ap, name):
            """LayerNorm (no affine) along free dim (768) of [128,768] fp32."""
            stats = tmpp.tile([128, 3, 6], F32, name=f"st_{name}", tag="ln_stats", bufs=2)
            mv = tmpp.tile([128, 2], F32, name=f"mv_{name}", tag="ln_mv", bufs=4)
            xg = x_ap.rearrange("p (sg d) -> p sg d", sg=3)
            for sg in range(3):
                nc.vector.bn_stats(out=stats[:, sg, :], in_=xg[:, sg, :])
            nc.vector.bn_aggr(out=mv[:], in_=stats[:])
            nc.scalar.activation(out=mv[:, 1:2], in_=mv[:, 1:2], func=ACT.Sqrt,
                                 bias=epst[:], scale=1.0)
            nc.vector.reciprocal(out=mv[:, 1:2], in_=mv[:, 1:2])
            nc.vector.tensor_scalar(out=x_ap, in0=x_ap, scalar1=mv[:, 0:1],
                                    scalar2=mv[:, 1:2],
                                    op0=ALU.subtract, op1=ALU.mult)

        def transpose_bf(dst_ap, src_bf_ap):
            """[128,128] bf16 transpose through the PE (1 cycle/row)."""
            pt = pstp.tile([128, 128], BF, name="pt", tag="tp")
            nc.tensor.transpose(pt[:], src_bf_ap, ident_bf[:])
            nc.vector.tensor_copy(out=dst_ap, in_=pt[:])

        def cast_transpose(dstT, src_f32, p, name):
            """f32 [128,768] token-major -> bf16 h-major dstT[:, :, p*128:...]."""
            xb = tmpp.tile([128, H], BF, name=f"cb_{name}", tag="castb", bufs=3)
            nc.scalar.copy(out=xb[:], in_=src_f32)
            for hh in range(HT):
                transpose_bf(dstT[:, hh, p * 128:(p + 1) * 128],
                             xb[:, hh * 128:(hh + 1) * 128])

        # ---- big-weight rotation: wqkv -> w1 -> w2 share 2 slots ----
        wqkv_sb = wbig.tile([128, HT, 3 * H], BF, name="wqkv_sb", tag="wbig")
        w1_sb = wbig.tile([128, HT, DFF], BF, name="w1_sb", tag="wbig")
        w2_sb = wbig.tile([128, FT, H], BF, name="w2_sb", tag="wbig")

        decA = tc.alloc_tile_pool(name="decA", bufs=1)

        # ================= DMA issue (sync queue: critical weight stream) ====
        xT_sb = decA.tile([128, HT, TOK], BF, name="xT_sb", tag="xT_sb")
        nc.sync.dma_start(out=xT_sb[:], in_=xT_d[:].rearrange("(ht p) t -> p ht t", p=128))
        wqkv_r = wqkvT_d[:].rearrange("(ht p) o -> p ht o", p=128)
        for (c0, cn) in CH[:2]:
            nc.sync.dma_start(out=wqkv_sb[:, :, c0:c0 + cn], in_=wqkv_r[:, :, c0:c0 + cn])
        wo_sb = decA.tile([128, HT, H], BF, name="wo_sb", tag="w_med", bufs=2)
        nc.sync.dma_start(out=wo_sb[:], in_=woT_d[:].rearrange("(ht p) o -> p ht o", p=128))
        for (c0, cn) in CH[2:]:
            nc.sync.dma_start(out=wqkv_sb[:, :, c0:c0 + cn], in_=wqkv_r[:, :, c0:c0 + cn])
        nc.sync.dma_start(out=w1_sb[:], in_=w1T_d[:].rearrange("(ht p) o -> p ht o", p=128))
        nc.sync.dma_start(out=w2_sb[:], in_=w2T_d[:].rearrange("(ft p) o -> p ft o", p=128))

        # ---- Act hwdge queue: secondary loads (CA path + token-major tgt) ----
        xtok_sb = decA.tile([128, NT, H], BF, name="xtok_sb", tag="xtok_sb")
        nc.scalar.dma_start(out=xtok_sb[:], in_=xtok_d[:])
        memT_sb = decA.tile([128, HT, S], BF, name="memT_sb", tag="memT_sb")
        nc.scalar.dma_start(out=memT_sb[:], in_=memT_d[:].rearrange("(ht p) s -> p ht s", p=128))
        wca_sb = decA.tile([128, HT, H], BF, name="wca_sb", tag="w_med", bufs=2)
        nc.scalar.dma_start(out=wca_sb[:], in_=wcaT_d[:].rearrange("(ht p) o -> p ht o", p=128))

        # ---- small broadcast tiles (gpsimd SWDGE) ----
        bqkv_bc = _bcast_load(nc, consts, bqkv_d, 3 * H, "bqkv_bc")
        bca_bc = _bcast_load(nc, consts, bca_d, H, "bca_bc")
        bres_bc = _bcast_load(nc, consts, bres_d, H, "bres_bc")
        g1_bc = _bcast_load(nc, consts, g1_d, H, "g1_bc")
        g2_bc = _bcast_load(nc, consts, g2_d, H, "g2_bc")
        b1_sb = consts.tile([128, FT], F32, name="b1_sb", tag="b1_sb")
        nc.gpsimd.dma_start(out=b1_sb[:], in_=b1_d[:].rearrange("(ft p) -> p ft", p=128))

        # ================= decoder compute =================
        qkv = decA.tile([128, NT, 3 * H], BF, name="qkv", tag="qkv")

        def qkv_chunk(ci, plist):
            c0, cn = CH[ci]
            for p in plist:
                if p == 0:
                    if c0 >= 768:
                        continue
                    cn_p = min(cn, 768 - c0)
                else:
                    cn_p = cn
                ps = psmm.tile([128, 512], F32, name="ps_qkv", tag="mm")[:, :cn_p]
                for h in range(HT):
                    nc.tensor.matmul(ps, xT_sb[:, h, p * 128:(p + 1) * 128],
                                     wqkv_sb[:, h, c0:c0 + cn_p],
                                     start=(h == 0), stop=(h == HT - 1))
                nc.vector.tensor_tensor(qkv[:, p, c0:c0 + cn_p], ps,
                                        bqkv_bc[:, c0:c0 + cn_p], ALU.add)

        # v/k chunks for everyone first (chunks 0,1), then CA, then q chunks
        qkv_chunk(0, [0, 1, 2])
        qkv_chunk(1, [0, 1, 2])

        # --- cross-attention: ONE folded matmul; + (bias + ln1_b) -> cab ---
        cab = decA.tile([128, H], F32, name="cab", tag="cab")
        for (c0, cn) in CH_H:
            ps = psmm.tile([128, 512], F32, name="ps_ca", tag="mm")[:, :cn]
            for h in range(HT):
                nc.tensor.matmul(ps, memT_sb[:, h, :], wca_sb[:, h, c0:c0 + cn],
                                 start=(h == 0), stop=(h == HT - 1))
            nc.vector.tensor_tensor(cab[:, c0:c0 + cn], ps, bca_bc[:, c0:c0 + cn], ALU.add)

        qkv_chunk(2, [1, 2])
        qkv_chunk(3, [1, 2])
        qkv_chunk(4, [1, 2])

        # --- per-position SA out-proj + residual + LN1 + (+cab) + LN2 ---
        oT = decA.tile([128, HT, TOK], BF, name="oT", tag="oT")
        x2h_t = {}

        def sa_ln12(p):
            x1p = scratch(f"x1_{p}")
            for (c0, cn) in CH_H:
                ps = psmm.tile([128, 512], F32, name="ps_sao", tag="mm")[:, :cn]
                for h in range(HT):
                    nc.tensor.matmul(ps, oT[:, h, p * 128:(p + 1) * 128],
                                     wo_sb[:, h, c0:c0 + cn],
                                     start=(h == 0), stop=(h == HT - 1))
                # residual (tgt + bo) folded into xtok upload
                nc.vector.tensor_tensor(x1p[:, c0:c0 + cn], ps,
                                        xtok_sb[:, p, c0:c0 + cn], ALU.add)
            ln_inplace(x1p[:], f"ln1_{p}")
            nc.vector.tensor_tensor(x1p[:], x1p[:], g1_bc[:, :], ALU.mult)
            nc.vector.tensor_tensor(x1p[:], x1p[:], cab[:], ALU.add)
            ln_inplace(x1p[:], f"ln2_{p}")
            x2h = tmpp.tile([128, H], F32, name=f"x2h_{p}", tag="x2h", bufs=3)
            nc.vector.tensor_copy(out=x2h[:], in_=x1p[:])
            x2h_t[p] = x2h
            cast_transpose(x2T, x1p[:], p, f"x2_{p}")

        # --- FFN pieces ---
        def lin1_p(p):
            h1p = ffnp.tile([128, FT, 128], BF, name=f"h1_{p}", tag="h1p", bufs=2)
            h1p_t[p] = h1p
            t0 = p * 128
            for ft in range(FT):
                ps = psmm.tile([128, 512], F32, name="ps_l1", tag="mm")[:, :128]
                for h in range(HT):
                    nc.tensor.matmul(ps, w1_sb[:, h, ft * 128:(ft + 1) * 128],
                                     x2T[:, h, t0:t0 + 128],
                                     start=(h == 0), stop=(h == HT - 1))
                nc.scalar.activation(out=h1p[:, ft, :], in_=ps, func=ACT.Relu,
                                     bias=b1_sb[:, ft:ft + 1], scale=1.0)

        def ffn_tail(p):
            # residual side: x2g = ln2hat * g2 + (lin2_b + ln2_b)   (gpsimd)
            x2g = x2h_t[p]
            nc.gpsimd.tensor_tensor(x2g[:], x2g[:], g2_bc[:, :], ALU.mult)
            nc.gpsimd.tensor_tensor(x2g[:], x2g[:], bres_bc[:, :], ALU.add)
            x3p = scratch(f"x3_{p}")
            for (c0, cn) in CH_H:
                ps = psmm.tile([128, 512], F32, name="ps_l2", tag="mm")[:, :cn]
                for ft in range(FT):
                    nc.tensor.matmul(ps, h1p_t[p][:, ft, :],
                                     w2_sb[:, ft, c0:c0 + cn],
                                     start=(ft == 0), stop=(ft == FT - 1))
                nc.vector.tensor_tensor(x3p[:, c0:c0 + cn], ps, x2g[:, c0:c0 + cn],
                                        ALU.add)
            ln_inplace(x3p[:], f"ln3_{p}")
            cast_transpose(x3T, x3p[:], p, f"x3_{p}")

        # ===== attention math for positions 1,2 (DVE; scores pre-scaled) =====
        w_t = {}

        def vheads(j):
            return qkv[:, j, 0:H].rearrange("p (nh hd) -> p nh hd", nh=NH)

        def wb(i, j):
            return w_t[i][:, j, :, None].to_broadcast((128, NH, HD))

        def attn(i):
            nj = i + 1
            s = decA.tile([128, 3, NH], F32, name=f"s{i}", tag=f"s{i}")[:, :nj, :]
            for j in range(nj):
                prod = scratch(f"prod{i}{j}")
                nc.vector.tensor_tensor(prod[:], qkv[:, i, 2 * H:3 * H],
                                        qkv[:, j, H:2 * H], ALU.mult)
                nc.vector.reduce_sum(out=s[:, j, :],
                                     in_=prod[:].rearrange("p (nh hd) -> p nh hd", nh=NH),
                                     axis=mybir.AxisListType.X)
            e = tmpp.tile([128, 3, NH], F32, name=f"e{i}", tag="sm_e", bufs=2)[:, :nj, :]
            nc.scalar.activation(out=e, in_=s, func=ACT.Exp)
            den = tmpp.tile([128, NH], F32, name=f"den{i}", tag="sm_small", bufs=8)
            nc.vector.reduce_sum(out=den[:], in_=e.rearrange("p j h -> p h j"),
                                 axis=mybir.AxisListType.X)
            nc.vector.reciprocal(out=den[:], in_=den[:])
            w = decA.tile([128, 3, NH], F32, name=f"w{i}", tag=f"w{i}")[:, :nj, :]
            nc.vector.tensor_tensor(w, e, den[:, None, :].to_broadcast((128, nj, NH)),
                                    ALU.mult)
            w_t[i] = w
            facc = scratch(f"facc{i}")
            tmp3 = scratch(f"tmp3{i}")
            fv = facc[:].rearrange("p (nh hd) -> p nh hd", nh=NH)
            tv = tmp3[:].rearrange("p (nh hd) -> p nh hd", nh=NH)
            nc.vector.tensor_tensor(fv, vheads(0), wb(i, 0), ALU.mult)
            nc.vector.tensor_tensor(tv, vheads(1), wb(i, 1), ALU.mult)
            if i == 1:
                o_i = scratch("o1")
                nc.vector.tensor_tensor(o_i[:], facc[:], tmp3[:], ALU.add)
            else:
                nc.vector.tensor_tensor(facc[:], facc[:], tmp3[:], ALU.add)
                nc.vector.tensor_tensor(tv, vheads(2), wb(2, 2), ALU.mult)
                o_i = scratch("o2")
                nc.vector.tensor_tensor(o_i[:], facc[:], tmp3[:], ALU.add)
            # o_i f32 -> bf16 (Act) -> PE transpose into oT
            ob = tmpp.tile([128, H], BF, name=f"ob{i}", tag="castb", bufs=3)
            nc.scalar.copy(out=ob[:], in_=o_i[:])
            for hh in range(HT):
                transpose_bf(oT[:, hh, i * 128:(i + 1) * 128],
                             ob[:, hh * 128:(hh + 1) * 128])

        # ================= vocab projection machinery =================
        projT_r = projT_d[:].rearrange("(ht p) v -> p ht v", p=128)
        wt_ring = {}

        def proj_load(job_idx, vg):
            wt = projp.tile([128, HT, VG], BF, name="wt", tag="projw")
            nc.sync.dma_start(out=wt[:], in_=projT_r[:, :, vg * VG:(vg + 1) * VG])
            wt_ring[job_idx] = wt

        pending_stores = []

        def flush_stores():
            for (dst, src) in pending_stores:
                nc.scalar.dma_start(out=dst, in_=src)
            pending_stores.clear()

        def proj_compute(job_idx, vg, plist, tail=False):
            wt = wt_ring.pop(job_idx)
            for p in plist:
                stg = stagep.tile([128, VG], BF, name=f"stg{p}", tag="stg", bufs=6)
                nhalf = VG // 512
                for half in range(nhalf):
                    hw_ = 160 if (vg == NVG - 1 and half == 1) else 512
                    ps = psmm.tile([128, 512], F32, name="ps_pr", tag="mm")[:, :hw_]
                    for h in range(HT):
                        nc.tensor.matmul(ps, x3T[:, h, p * 128:(p + 1) * 128],
                                         wt[:, h, half * 512:half * 512 + hw_],
                                         start=(h == 0), stop=(h == HT - 1))
                    dst = stg[:, half * 512:half * 512 + hw_]
                    if half % 2 == 0:
                        nc.vector.tensor_copy(out=dst, in_=ps)
                    else:
                        nc.scalar.copy(out=dst, in_=ps)
                    if tail:  # flush per-half to shrink the exit tail
                        nc.scalar.dma_start(
                            out=out_d[:, p, vg * VG + half * 512: vg * VG + half * 512 + hw_],
                            in_=stg[:, half * 512:half * 512 + hw_])
                if not tail:
                    # deferred: emitted at the next job so the store's wait
                    # never blocks the Act sequencer behind unmet deps
                    pending_stores.append((out_d[:, p, vg * VG:(vg + 1) * VG],
                                           stg[:]))

        # ================= emission schedule =================
        # p0 fast path
        for hh in range(HT):   # o(p0) = v0 (already bf16 in qkv)
            transpose_bf(oT[:, hh, 0:128],
                         qkv[:, 0, hh * 128:(hh + 1) * 128])
        sa_ln12(0)
        lin1_p(0)
        ffn_tail(0)            # -> x3T p0 ready

        attn(1)                # DVE chain for p1
        attn(2)

        # projection jobs interleaved with the p1/p2 decoder tail.
        # job list: (vg, plist) in emission order; re-streams at the end.
        jobs = []
        for vg in range(K0):
            jobs.append((vg, [0]))
        for vg in range(K0, K1):
            jobs.append((vg, [0, 1]))
        for vg in range(K1, NVG):
            jobs.append((vg, [0, 1, 2]))
        for vg in range(K0):
            jobs.append((vg, [1, 2]))
        for vg in range(K0, K1):
            jobs.append((vg, [2]))

        # decoder-tail emission points: before job n, emit the decoder step
        tail_steps = {
            0: lambda: sa_ln12(1),
            1: lambda: lin1_p(1),
            2: lambda: ffn_tail(1),
            3: lambda: sa_ln12(2),
            4: lambda: lin1_p(2),
            5: lambda: ffn_tail(2),
        }

        proj_load(0, jobs[0][0])
        proj_load(1, jobs[1][0])
        for n, (vg, plist) in enumerate(jobs):
            if n + 2 < len(jobs):
                proj_load(n + 2, jobs[n + 2][0])
            step = tail_steps.get(n)
            if step is not None:
                step()
            flush_stores()
            proj_compute(n, vg, plist, tail=(n >= len(jobs) - 2))
        flush_stores()

        decA.release()
        pstp.release()
        psmm.release()
        ffnp.release()
        wbig.release()
        tmpp.release()
        stagep.release()
        projp.release()
        longl.release()
        consts.release()

    nc.finalize()
    return nc


_NC_CACHE = None


def _get_nc():
    global _NC_CACHE
    if _NC_CACHE is None:
        _NC_CACHE = build_program()
    return _NC_CACHE


_B3VEC = None


def _prep_inputs(inputs):
    global _B3VEC
    f32 = np.float32
    enc = np.asarray(inputs["encoder_hidden"], f32)           # (B,T,H)
    tok = np.asarray(inputs["teacher_tokens"]).astype(np.int64)
    emb = np.asarray(inputs["emb"], f32)
    start = np.asarray(inputs["start_token"], f32)
    N = B * T

    tgt = np.empty((N, NT, H), f32)
    tgt[:, 0, :] = start.reshape(1, H)
    tgt[:, 1:, :] = emb[tok.reshape(N, NT)[:, : NT - 1]]
    mem = enc.reshape(N, H)

    def bfc(a):
        return np.ascontiguousarray(np.asarray(a, dtype=f32)).astype(BF16)

    # ---- host-side weight algebra (folds) ----
    wq = np.asarray(inputs["sa_in_w"], f32)[0:H]        # (H,H): q = x @ wq.T
    wk = np.asarray(inputs["sa_in_w"], f32)[H:2 * H]
    wv = np.asarray(inputs["sa_in_w"], f32)[2 * H:]
    bq = np.asarray(inputs["sa_in_b"], f32)[0:H]
    bk = np.asarray(inputs["sa_in_b"], f32)[H:2 * H]
    bv = np.asarray(inputs["sa_in_b"], f32)[2 * H:]
    c_inv = 1.0 / np.sqrt(np.float32(HD))
    # packed [v|k|q], 1/sqrt(hd) folded into q
    wqkv_packed = np.concatenate([wv, wk, wq * c_inv], axis=0)       # (3H, H)
    bqkv_packed = np.concatenate([bv, bk, bq * c_inv], axis=0)

    ca_wv = np.asarray(inputs["ca_in_w"], f32)[2 * H:]  # (H,H)
    ca_bv = np.asarray(inputs["ca_in_b"], f32)[2 * H:]
    ca_wo = np.asarray(inputs["ca_out_w"], f32)         # (H,H)
    ca_bo = np.asarray(inputs["ca_out_b"], f32)
    # ca(mem) = mem @ (ca_wv.T @ ca_wo.T) + (ca_bv @ ca_wo.T + ca_bo)
    wca = ca_wv.T @ ca_wo.T                             # (H,H) input-major
    bca = ca_bv @ ca_wo.T + ca_bo + np.asarray(inputs["ln1_b"], f32)

    g2 = np.asarray(inputs["ln2_g"], f32)
    b2 = np.asarray(inputs["ln2_b"], f32)
    w1 = np.asarray(inputs["lin1_w"], f32)              # (DFF,H)
    w1f = w1 * g2[None, :]
    b1f = np.asarray(inputs["lin1_b"], f32) + w1 @ b2
    bres = np.asarray(inputs["lin2_b"], f32) + b2

    g3 = np.asarray(inputs["ln3_g"], f32)
    b3 = np.asarray(inputs["ln3_b"], f32)
    projw = np.asarray(inputs["proj_w"], f32)           # (V,H)
    projf = projw * g3[None, :]
    _B3VEC = (projw @ b3).astype(f32)                   # (V,) host-added bias

    shared = {
        "wqkvT": bfc(wqkv_packed.T),
        "woT": bfc(np.asarray(inputs["sa_out_w"], f32).T),
        "wcaT": bfc(wca),
        "w1T": bfc(w1f.T),
        "w2T": bfc(np.asarray(inputs["lin2_w"], f32).T),
        "bqkv": bfc(bqkv_packed),
        "bca": bfc(bca),
        "bres": bfc(bres),
        "b1": b1f.astype(f32),
        "g1": bfc(inputs["ln1_g"]),
        "g2": bfc(g2),
    }
    projT = np.zeros((H, VP), BF16)
    projT[:, :V] = projf.T.astype(BF16)
    shared["projT"] = projT

    bo = np.asarray(inputs["sa_out_b"], f32)
    tgt_bo = tgt + bo.reshape(1, 1, H)                  # residual + out bias

    in_maps = []
    for c in range(NCORES):
        sl = slice(c * S, (c + 1) * S)
        tgt_c = tgt[sl]                                       # (128,3,768)
        m = dict(shared)
        m["xT"] = np.ascontiguousarray(
            tgt_c.transpose(2, 1, 0).reshape(H, TOK)).astype(BF16)     # (768,384)
        m["xtok"] = np.ascontiguousarray(tgt_bo[sl]).astype(BF16)      # (128,3,768)
        m["memT"] = np.ascontiguousarray(mem[sl].T).astype(BF16)       # (768,128)
        in_maps.append(m)
    return in_maps


def kernel(**inputs):
    nc = _get_nc()
    in_maps = _prep_inputs(inputs)
    res = run_bass_kernel_spmd(nc, in_maps, core_ids=list(range(NCORES)))
    final = np.empty((B * T, NT, V), np.float32)
    for c in range(NCORES):
        final[c * S:(c + 1) * S] = res.results[c]["out"][:, :, :V].astype(np.float32)
    final += _B3VEC.reshape(1, 1, V)
    return final.reshape(B, T, NT, V)


# revision 10
# speedup vs baseline: 1.0014x; 1.0014x over previous
"""Trainium2 Bass kernel for nn_CausalMultiTokenPredictionHead.

Distribution: pure data parallel over the flattened B*T axis (1024 sequences
-> 128 per core x 8 cores). Each core runs the full 3-token causal decoder
layer for its 128 sequences and projects its 384 tokens against the full
(padded) vocab. Decoder weights + the vocab projection table are replicated.

Math notes (exact simplifications, no approximations beyond bf16 rounding):
  - Cross-attention has memory length 1 -> softmax over a single key is
    identically 1, so ca(x) = out_proj(v_proj(mem)) independent of x. The two
    projections collapse into ONE host-folded [H,H] matrix; the ln1 beta is
    folded into its bias.
  - Self-attention is over 3 tokens with a causal mask -> per-position
    closed-form softmax over <=3 scores on the vector engine. Scores are
    bounded (|s| < ~3) so the max-subtraction is skipped; 1/sqrt(hd) is
    folded into Wq on the host.
  - Position 0 attends only to itself, so o(p0) = v0; the kernel pushes
    position 0 through the decoder first so the vocab projection can start
    ~25us in and fill the tensor engine while positions 1/2 finish. The
    first few vocab weight groups are re-streamed later for p1/p2.
  - qkv is packed [v|k|q] so position 0 only computes its v columns.
  - LN2's affine folds into lin1_w/lin1_b (residual side applies g2 on DVE);
    LN3's affine folds into proj_w, with the beta3 @ proj_w^T vector added
    on the host after download (it is all-zeros in practice).
  - The tgt residual + sa_out bias are folded into a token-major upload of
    the target embeddings.
All matmuls run in bf16 (fp32 PSUM accumulation); layernorms, softmax and
the residual stream are fp32. All PE transposes run in bf16 (1 cycle/row).
Logits are written to DRAM as bf16 and upcast on the host.
"""
import numpy as np
import ml_dtypes

import concourse.bass as bass
import concourse.mybir as mybir
import concourse.tile as tile
from concourse import bacc
from concourse.bass_utils import run_bass_kernel_spmd
from concourse.masks import make_identity

BF16 = ml_dtypes.bfloat16
F32 = mybir.dt.float32
BF = mybir.dt.bfloat16

B, T, H, V, NT, NH, DFF = 2, 512, 768, 51865, 3, 4, 2048
EPS = 1e-5
NCORES = 8
S = 128                       # sequences per core
TOK = S * NT                  # tokens per core (pos-major: t = p*128 + s)
HT = H // 128                 # 6 h-tiles
FT = DFF // 128               # 16 dff-tiles
HD = H // NH                  # 192 head dim
VP = 52224                    # padded vocab (102 * 512)
VG = 1024                     # vocab columns per streamed weight group
NVG = VP // VG                # 51 groups
ACT = mybir.ActivationFunctionType
ALU = mybir.AluOpType

# packed qkv column chunks (<=512 for one PSUM bank); layout [v|k|q]
CH = [(0, 512), (512, 512), (1024, 512), (1536, 512), (2048, 256)]
CH0 = [(0, 512), (512, 256)]   # p0 only needs v (cols 0:768)
CH_H = [(0, 512), (512, 256)]  # 768 split

# projection job schedule: how many leading vocab groups run before
# x3T[p1] / x3T[p2] are ready (those groups are re-streamed later).
K0 = 2   # groups done p0-only
K1 = 5   # groups done p0+p1
NWARM = 110  # PE warm-up dummy transposes (hold the p-state ramp at start)


def _bcast_load(nc, pool, dram, n, name, dtype=BF):
    """[n] DRAM vector -> [128, n] SBUF tile broadcast across partitions."""
    t = pool.tile([128, n], dtype, name=name, tag=name)
    ap = dram[:]
    bc = bass.AP(tensor=ap.tensor, offset=ap.offset, ap=[[0, 128]] + list(ap.ap))
    nc.gpsimd.dma_start(out=t[:], in_=bc)
    return t


def build_program():
    nc = bacc.Bacc(None, target_bir_lowering=False)

    # ---- DRAM I/O ----
    xT_d = nc.dram_tensor("xT", [H, TOK], BF, kind="ExternalInput")
    xtok_d = nc.dram_tensor("xtok", [S, NT, H], BF, kind="ExternalInput")
    memT_d = nc.dram_tensor("memT", [H, S], BF, kind="ExternalInput")
    wqkvT_d = nc.dram_tensor("wqkvT", [H, 3 * H], BF, kind="ExternalInput")
    woT_d = nc.dram_tensor("woT", [H, H], BF, kind="ExternalInput")
    wcaT_d = nc.dram_tensor("wcaT", [H, H], BF, kind="ExternalInput")
    w1T_d = nc.dram_tensor("w1T", [H, DFF], BF, kind="ExternalInput")
    w2T_d = nc.dram_tensor("w2T", [DFF, H], BF, kind="ExternalInput")
    projT_d = nc.dram_tensor("projT", [H, VP], BF, kind="ExternalInput")
    bqkv_d = nc.dram_tensor("bqkv", [3 * H], BF, kind="ExternalInput")
    bca_d = nc.dram_tensor("bca", [H], BF, kind="ExternalInput")
    bres_d = nc.dram_tensor("bres", [H], BF, kind="ExternalInput")
    b1_d = nc.dram_tensor("b1", [DFF], F32, kind="ExternalInput")
    g1_d = nc.dram_tensor("g1", [H], BF, kind="ExternalInput")
    g2_d = nc.dram_tensor("g2", [H], BF, kind="ExternalInput")
    out_d = nc.dram_tensor("out", [S, NT, VP], BF, kind="ExternalOutput")

    with tile.TileContext(nc) as tc:
        consts = tc.alloc_tile_pool(name="consts", bufs=1)
        longl = tc.alloc_tile_pool(name="longl", bufs=1)
        projp = tc.alloc_tile_pool(name="projp", bufs=3)
        stagep = tc.alloc_tile_pool(name="stagep", bufs=3)
        tmpp = tc.alloc_tile_pool(name="tmpp", bufs=1)
        wbig = tc.alloc_tile_pool(name="wbig", bufs=2)
        ffnp = tc.alloc_tile_pool(name="ffnp", bufs=1)
        psmm = tc.alloc_tile_pool(name="psmm", bufs=6, space="PSUM")
        pstp = tc.alloc_tile_pool(name="pstp", bufs=2, space="PSUM")

        # ---- constants ----
        ident_bf = consts.tile([128, 128], BF, name="ident_bf", tag="ident_bf")
        make_identity(nc, ident_bf)
        epst = consts.tile([128, 1], F32, name="epst", tag="epst")
        nc.vector.memset(epst, EPS)

        # ---- long-lived activations ----
        x3T = longl.tile([128, HT, TOK], BF, name="x3T", tag="x3T")
        x2T = longl.tile([128, HT, TOK], BF, name="x2T", tag="x2T")
        h1p_t = {}

        def scratch(name):
            return tmpp.tile([128, H], F32, name=name, tag="scratch", bufs=3)

        def ln_inplace(x_ap, name):
            """LayerNorm (no affine) along free dim (768) of [128,768] fp32."""
            stats = tmpp.tile([128, 3, 6], F32, name=f"st_{name}", tag="ln_stats", bufs=2)
            mv = tmpp.tile([128, 2], F32, name=f"mv_{name}", tag="ln_mv", bufs=4)
            xg = x_ap.rearrange("p (sg d) -> p sg d", sg=3)
            for sg in range(3):
                nc.vector.bn_stats(out=stats[:, sg, :], in_=xg[:, sg, :])
            nc.vector.bn_aggr(out=mv[:], in_=stats[:])
            nc.scalar.activation(out=mv[:, 1:2], in_=mv[:, 1:2], func=ACT.Sqrt,
                                 bias=epst[:], scale=1.0)
            nc.vector.reciprocal(out=mv[:, 1:2], in_=mv[:, 1:2])
            nc.vector.tensor_scalar(out=x_ap, in0=x_ap, scalar1=mv[:, 0:1],
                                    scalar2=mv[:, 1:2],
                                    op0=ALU.subtract, op1=ALU.mult)

        def transpose_bf(dst_ap, src_bf_ap):
            """[128,128] bf16 transpose through the PE (1 cycle/row)."""
            pt = pstp.tile([128, 128], BF, name="pt", tag="tp")
            nc.tensor.transpose(pt[:], src_bf_ap, ident_bf[:])
            nc.vector.tensor_copy(out=dst_ap, in_=pt[:])

        def cast_transpose(dstT, src_f32, p, name):
            """f32 [128,768] token-major -> bf16 h-major dstT[:, :, p*128:...]."""
            xb = tmpp.tile([128, H], BF, name=f"cb_{name}", tag="castb", bufs=3)
            nc.scalar.copy(out=xb[:], in_=src_f32)
            for hh in range(HT):
                transpose_bf(dstT[:, hh, p * 128:(p + 1) * 128],
                             xb[:, hh * 128:(hh + 1) * 128])

        # ---- big-weight rotation: wqkv -> w1 -> w2 share 2 slots ----
        wqkv_sb = wbig.tile([128, HT, 3 * H], BF, name="wqkv_sb", tag="wbig")
        w1_sb = wbig.tile([128, HT, DFF], BF, name="w1_sb", tag="wbig")
        w2_sb = wbig.tile([128, FT, H], BF, name="w2_sb", tag="wbig")

        decA = tc.alloc_tile_pool(name="decA", bufs=1)

        # ============ DMA issue: ONE queue (sync), strict priority order ====
        # need-times: xT/v-chunks (qkv p0) < wo/xtok (sa p0) < memT/wca (CA)
        # < w1 head (lin1 p0) < k/q chunks (attn p1/p2) < w1 tail < w2 < proj
        xT_sb = decA.tile([128, HT, TOK], BF, name="xT_sb", tag="xT_sb")
        nc.sync.dma_start(out=xT_sb[:], in_=xT_d[:].rearrange("(ht p) t -> p ht t", p=128))
        wqkv_r = wqkvT_d[:].rearrange("(ht p) o -> p ht o", p=128)
        for (c0, cn) in CH[:2]:
            nc.sync.dma_start(out=wqkv_sb[:, :, c0:c0 + cn], in_=wqkv_r[:, :, c0:c0 + cn])
        wo_sb = decA.tile([128, HT, H], BF, name="wo_sb", tag="w_med", bufs=2)
        nc.sync.dma_start(out=wo_sb[:], in_=woT_d[:].rearrange("(ht p) o -> p ht o", p=128))
        xtok_sb = decA.tile([128, NT, H], BF, name="xtok_sb", tag="xtok_sb")
        nc.sync.dma_start(out=xtok_sb[:], in_=xtok_d[:])
        memT_sb = decA.tile([128, HT, S], BF, name="memT_sb", tag="memT_sb")
        nc.sync.dma_start(out=memT_sb[:], in_=memT_d[:].rearrange("(ht p) s -> p ht s", p=128))
        wca_sb = decA.tile([128, HT, H], BF, name="wca_sb", tag="w_med", bufs=2)
        nc.sync.dma_start(out=wca_sb[:], in_=wcaT_d[:].rearrange("(ht p) o -> p ht o", p=128))
        w1_r = w1T_d[:].rearrange("(ht p) o -> p ht o", p=128)
        nc.sync.dma_start(out=w1_sb[:, :, 0:512], in_=w1_r[:, :, 0:512])
        for (c0, cn) in CH[2:]:
            nc.sync.dma_start(out=wqkv_sb[:, :, c0:c0 + cn], in_=wqkv_r[:, :, c0:c0 + cn])
        for c0 in range(512, DFF, 512):
            nc.sync.dma_start(out=w1_sb[:, :, c0:c0 + 512], in_=w1_r[:, :, c0:c0 + 512])
        w2_r = w2T_d[:].rearrange("(ft p) o -> p ft o", p=128)
        for f0 in range(0, FT, 4):
            nc.sync.dma_start(out=w2_sb[:, f0:f0 + 4, :], in_=w2_r[:, f0:f0 + 4, :])

        # ---- small broadcast tiles (gpsimd SWDGE) ----
        bqkv_bc = _bcast_load(nc, consts, bqkv_d, 3 * H, "bqkv_bc")
        bca_bc = _bcast_load(nc, consts, bca_d, H, "bca_bc")
        g1_bc = _bcast_load(nc, consts, g1_d, H, "g1_bc")
        bres_bc = _bcast_load(nc, consts, bres_d, H, "bres_bc")
        g2_bc = _bcast_load(nc, consts, g2_d, H, "g2_bc")
        b1_sb = consts.tile([128, FT], F32, name="b1_sb", tag="b1_sb")
        nc.gpsimd.dma_start(out=b1_sb[:], in_=b1_d[:].rearrange("(ft p) -> p ft", p=128))

        # ---- PE warm-up: dummy transposes keep the p-state ramp hot while
        # the first weight DMAs are in flight (PE would be idle anyway).
        for _ in range(NWARM):
            pt = pstp.tile([128, 128], BF, name="pt", tag="tp")
            nc.tensor.transpose(pt[:], ident_bf[:], ident_bf[:])

        # ================= decoder compute =================
        qkv = decA.tile([128, NT, 3 * H], BF, name="qkv", tag="qkv")

        def qkv_chunk(ci, plist):
            c0, cn = CH[ci]
            for p in plist:
                if p == 0:
                    if c0 >= 768:
                        continue
                    cn_p = min(cn, 768 - c0)
                else:
                    cn_p = cn
                ps = psmm.tile([128, 512], F32, name="ps_qkv", tag="mm")[:, :cn_p]
                for h in range(HT):
                    nc.tensor.matmul(ps, xT_sb[:, h, p * 128:(p + 1) * 128],
                                     wqkv_sb[:, h, c0:c0 + cn_p],
                                     start=(h == 0), stop=(h == HT - 1))
                nc.vector.tensor_tensor(qkv[:, p, c0:c0 + cn_p], ps,
                                        bqkv_bc[:, c0:c0 + cn_p], ALU.add)

        # v/k chunks for everyone first (chunks 0,1), then CA, then q chunks
        qkv_chunk(0, [0, 1, 2])
        qkv_chunk(1, [0, 1, 2])

        # --- cross-attention: ONE folded matmul; + (bias + ln1_b) -> cab ---
        cab = decA.tile([128, H], F32, name="cab", tag="cab")
        for (c0, cn) in CH_H:
            ps = psmm.tile([128, 512], F32, name="ps_ca", tag="mm")[:, :cn]
            for h in range(HT):
                nc.tensor.matmul(ps, memT_sb[:, h, :], wca_sb[:, h, c0:c0 + cn],
                                 start=(h == 0), stop=(h == HT - 1))
            nc.vector.tensor_tensor(cab[:, c0:c0 + cn], ps, bca_bc[:, c0:c0 + cn], ALU.add)

        qkv_chunk(2, [1, 2])
        qkv_chunk(3, [1, 2])
        qkv_chunk(4, [1, 2])

        # --- per-position SA out-proj + residual + LN1 + (+cab) + LN2 ---
        oT = decA.tile([128, HT, TOK], BF, name="oT", tag="oT")
        x2h_t = {}

        def sa_ln12(p):
            x1p = tmpp.tile([128, H], F32, name=f"x1_{p}", tag="x1p", bufs=3)
            for (c0, cn) in CH_H:
                ps = psmm.tile([128, 512], F32, name="ps_sao", tag="mm")[:, :cn]
                for h in range(HT):
                    nc.tensor.matmul(ps, oT[:, h, p * 128:(p + 1) * 128],
                                     wo_sb[:, h, c0:c0 + cn],
                                     start=(h == 0), stop=(h == HT - 1))
                # residual (tgt + bo) folded into xtok upload
                nc.vector.tensor_tensor(x1p[:, c0:c0 + cn], ps,
                                        xtok_sb[:, p, c0:c0 + cn], ALU.add)
            ln_inplace(x1p[:], f"ln1_{p}")
            nc.vector.tensor_tensor(x1p[:], x1p[:], g1_bc[:, :], ALU.mult)
            nc.vector.tensor_tensor(x1p[:], x1p[:], cab[:], ALU.add)
            ln_inplace(x1p[:], f"ln2_{p}")
            x2h_t[p] = x1p
            cast_transpose(x2T, x1p[:], p, f"x2_{p}")

        # --- FFN pieces ---
        def lin1_p(p):
            h1p = ffnp.tile([128, FT, 128], BF, name=f"h1_{p}", tag="h1p", bufs=2)
            h1p_t[p] = h1p
            t0 = p * 128
            for ft in range(FT):
                ps = psmm.tile([128, 512], F32, name="ps_l1", tag="mm")[:, :128]
                for h in range(HT):
                    nc.tensor.matmul(ps, w1_sb[:, h, ft * 128:(ft + 1) * 128],
                                     x2T[:, h, t0:t0 + 128],
                                     start=(h == 0), stop=(h == HT - 1))
                nc.scalar.activation(out=h1p[:, ft, :], in_=ps, func=ACT.Relu,
                                     bias=b1_sb[:, ft:ft + 1], scale=1.0)

        def ffn_tail(p):
            # residual side: x2g = ln2hat * g2 + (lin2_b + ln2_b)   (gpsimd)
            x2g = x2h_t[p]
            nc.gpsimd.tensor_tensor(x2g[:], x2g[:], g2_bc[:, :], ALU.mult)
            nc.gpsimd.tensor_tensor(x2g[:], x2g[:], bres_bc[:, :], ALU.add)
            x3p = scratch(f"x3_{p}")
            for (c0, cn) in CH_H:
                ps = psmm.tile([128, 512], F32, name="ps_l2", tag="mm")[:, :cn]
                for ft in range(FT):
                    nc.tensor.matmul(ps, h1p_t[p][:, ft, :],
                                     w2_sb[:, ft, c0:c0 + cn],
                                     start=(ft == 0), stop=(ft == FT - 1))
                nc.vector.tensor_tensor(x3p[:, c0:c0 + cn], ps, x2g[:, c0:c0 + cn],
                                        ALU.add)
            ln_inplace(x3p[:], f"ln3_{p}")
            cast_transpose(x3T, x3p[:], p, f"x3_{p}")

        # ===== attention math for positions 1,2 (DVE; scores pre-scaled) =====
        w_t = {}

        def vheads(j):
            return qkv[:, j, 0:H].rearrange("p (nh hd) -> p nh hd", nh=NH)

        def wb(i, j):
            return w_t[i][:, j, :, None].to_broadcast((128, NH, HD))

        def attn(i):
            # the [768]-wide ops for position 2 run on the (idle) Pool engine
            # so the p1 and p2 chains overlap; tiny softmax ops stay on DVE.
            eng = nc.gpsimd if i == 2 else nc.vector
            nj = i + 1
            s = decA.tile([128, 3, NH], F32, name=f"s{i}", tag=f"s{i}")[:, :nj, :]
            for j in range(nj):
                prod = scratch(f"prod{i}{j}")
                eng.tensor_tensor(prod[:], qkv[:, i, 2 * H:3 * H],
                                  qkv[:, j, H:2 * H], ALU.mult)
                nc.vector.reduce_sum(out=s[:, j, :],
                                     in_=prod[:].rearrange("p (nh hd) -> p nh hd", nh=NH),
                                     axis=mybir.AxisListType.X)
            e = tmpp.tile([128, 3, NH], F32, name=f"e{i}", tag="sm_e", bufs=2)[:, :nj, :]
            nc.scalar.activation(out=e, in_=s, func=ACT.Exp)
            den = tmpp.tile([128, NH], F32, name=f"den{i}", tag="sm_small", bufs=8)
            nc.vector.reduce_sum(out=den[:], in_=e.rearrange("p j h -> p h j"),
                                 axis=mybir.AxisListType.X)
            nc.vector.reciprocal(out=den[:], in_=den[:])
            w = decA.tile([128, 3, NH], F32, name=f"w{i}", tag=f"w{i}")[:, :nj, :]
            nc.vector.tensor_tensor(w, e, den[:, None, :].to_broadcast((128, nj, NH)),
                                    ALU.mult)
            w_t[i] = w
            facc = scratch(f"facc{i}")
            tmp3 = scratch(f"tmp3{i}")
            fv = facc[:].rearrange("p (nh hd) -> p nh hd", nh=NH)
            tv = tmp3[:].rearrange("p (nh hd) -> p nh hd", nh=NH)
            eng.tensor_tensor(fv, vheads(0), wb(i, 0), ALU.mult)
            eng.tensor_tensor(tv, vheads(1), wb(i, 1), ALU.mult)
            if i == 1:
                o_i = scratch("o1")
                eng.tensor_tensor(o_i[:], facc[:], tmp3[:], ALU.add)
            else:
                eng.tensor_tensor(facc[:], facc[:], tmp3[:], ALU.add)
                eng.tensor_tensor(tv, vheads(2), wb(2, 2), ALU.mult)
                o_i = scratch("o2")
                eng.tensor_tensor(o_i[:], facc[:], tmp3[:], ALU.add)
            # o_i f32 -> bf16 (Act) -> PE transpose into oT
            ob = tmpp.tile([128, H], BF, name=f"ob{i}", tag="castb", bufs=3)
            nc.scalar.copy(out=ob[:], in_=o_i[:])
            for hh in range(HT):
                transpose_bf(oT[:, hh, i * 128:(i + 1) * 128],
                             ob[:, hh * 128:(hh + 1) * 128])

        # ================= vocab projection machinery =================
        projT_r = projT_d[:].rearrange("(ht p) v -> p ht v", p=128)
        wt_ring = {}

        def proj_load(job_idx, vg):
            wt = projp.tile([128, HT, VG], BF, name="wt", tag="projw")
            nc.sync.dma_start(out=wt[:], in_=projT_r[:, :, vg * VG:(vg + 1) * VG])
            wt_ring[job_idx] = wt

        pending_stores = []

        def flush_stores():
            for (dst, src) in pending_stores:
                nc.scalar.dma_start(out=dst, in_=src)
            pending_stores.clear()

        def proj_compute(job_idx, vg, plist, tail=False):
            wt = wt_ring.pop(job_idx)
            for p in plist:
                stg = stagep.tile([128, VG], BF, name=f"stg{p}", tag="stg", bufs=6)
                nhalf = VG // 512
                for half in range(nhalf):
                    hw_ = 160 if (vg == NVG - 1 and half == 1) else 512
                    ps = psmm.tile([128, 512], F32, name="ps_pr", tag="mm")[:, :hw_]
                    for h in range(HT):
                        nc.tensor.matmul(ps, x3T[:, h, p * 128:(p + 1) * 128],
                                         wt[:, h, half * 512:half * 512 + hw_],
                                         start=(h == 0), stop=(h == HT - 1))
                    dst = stg[:, half * 512:half * 512 + hw_]
                    if half % 2 == 0:
                        nc.vector.tensor_copy(out=dst, in_=ps)
                    else:
                        nc.scalar.copy(out=dst, in_=ps)
                    if tail:  # flush per-half to shrink the exit tail
                        nc.scalar.dma_start(
                            out=out_d[:, p, vg * VG + half * 512: vg * VG + half * 512 + hw_],
                            in_=stg[:, half * 512:half * 512 + hw_])
                if not tail:
                    # deferred: emitted at the next job so the store's wait
                    # never blocks the Act sequencer behind unmet deps
                    pending_stores.append((out_d[:, p, vg * VG:(vg + 1) * VG],
                                           stg[:]))

        # ================= emission schedule =================
        # p0 fast path
        for hh in range(HT):   # o(p0) = v0 (already bf16 in qkv)
            transpose_bf(oT[:, hh, 0:128],
                         qkv[:, 0, hh * 128:(hh + 1) * 128])
        sa_ln12(0)
        lin1_p(0)
        ffn_tail(0)            # -> x3T p0 ready

        attn(1)                # DVE chain for p1
        attn(2)

        # projection jobs interleaved with the p1/p2 decoder tail.
        # job list: (vg, plist) in emission order; re-streams at the end.
        jobs = []
        for vg in range(K0):
            jobs.append((vg, [0]))
        for vg in range(K0, K1):
            jobs.append((vg, [0, 1]))
        for vg in range(K1, NVG):
            jobs.append((vg, [0, 1, 2]))
        for vg in range(K0):
            jobs.append((vg, [1, 2]))
        for vg in range(K0, K1):
            jobs.append((vg, [2]))

        # decoder-tail emission points: before job n, emit the decoder step
        tail_steps = {
            0: lambda: sa_ln12(1),
            1: lambda: lin1_p(1),
            2: lambda: ffn_tail(1),
            3: lambda: sa_ln12(2),
            4: lambda: lin1_p(2),
            5: lambda: ffn_tail(2),
        }

        proj_load(0, jobs[0][0])
        proj_load(1, jobs[1][0])
        for n, (vg, plist) in enumerate(jobs):
            if n + 2 < len(jobs):
                proj_load(n + 2, jobs[n + 2][0])
            step = tail_steps.get(n)
            if step is not None:
                step()
            flush_stores()
            proj_compute(n, vg, plist, tail=(n >= len(jobs) - 2))
        flush_stores()

        decA.release()
        pstp.release()
        psmm.release()
        ffnp.release()
        wbig.release()
        tmpp.release()
        stagep.release()
        projp.release()
        longl.release()
        consts.release()

    nc.finalize()
    return nc


_NC_CACHE = None


def _get_nc():
    global _NC_CACHE
    if _NC_CACHE is None:
        _NC_CACHE = build_program()
    return _NC_CACHE


_B3VEC = None


def _prep_inputs(inputs):
    global _B3VEC
    f32 = np.float32
    enc = np.asarray(inputs["encoder_hidden"], f32)           # (B,T,H)
    tok = np.asarray(inputs["teacher_tokens"]).astype(np.int64)
    emb = np.asarray(inputs["emb"], f32)
    start = np.asarray(inputs["start_token"], f32)
    N = B * T

    tgt = np.empty((N, NT, H), f32)
    tgt[:, 0, :] = start.reshape(1, H)
    tgt[:, 1:, :] = emb[tok.reshape(N, NT)[:, : NT - 1]]
    mem = enc.reshape(N, H)

    def bfc(a):
        return np.ascontiguousarray(np.asarray(a, dtype=f32)).astype(BF16)

    # ---- host-side weight algebra (folds) ----
    wq = np.asarray(inputs["sa_in_w"], f32)[0:H]        # (H,H): q = x @ wq.T
    wk = np.asarray(inputs["sa_in_w"], f32)[H:2 * H]
    wv = np.asarray(inputs["sa_in_w"], f32)[2 * H:]
    bq = np.asarray(inputs["sa_in_b"], f32)[0:H]
    bk = np.asarray(inputs["sa_in_b"], f32)[H:2 * H]
    bv = np.asarray(inputs["sa_in_b"], f32)[2 * H:]
    c_inv = 1.0 / np.sqrt(np.float32(HD))
    # packed [v|k|q], 1/sqrt(hd) folded into q
    wqkv_packed = np.concatenate([wv, wk, wq * c_inv], axis=0)       # (3H, H)
    bqkv_packed = np.concatenate([bv, bk, bq * c_inv], axis=0)

    ca_wv = np.asarray(inputs["ca_in_w"], f32)[2 * H:]  # (H,H)
    ca_bv = np.asarray(inputs["ca_in_b"], f32)[2 * H:]
    ca_wo = np.asarray(inputs["ca_out_w"], f32)         # (H,H)
    ca_bo = np.asarray(inputs["ca_out_b"], f32)
    # ca(mem) = mem @ (ca_wv.T @ ca_wo.T) + (ca_bv @ ca_wo.T + ca_bo)
    wca = ca_wv.T @ ca_wo.T                             # (H,H) input-major
    bca = ca_bv @ ca_wo.T + ca_bo + np.asarray(inputs["ln1_b"], f32)

    g2 = np.asarray(inputs["ln2_g"], f32)
    b2 = np.asarray(inputs["ln2_b"], f32)
    w1 = np.asarray(inputs["lin1_w"], f32)              # (DFF,H)
    w1f = w1 * g2[None, :]
    b1f = np.asarray(inputs["lin1_b"], f32) + w1 @ b2
    bres = np.asarray(inputs["lin2_b"], f32) + b2

    g3 = np.asarray(inputs["ln3_g"], f32)
    b3 = np.asarray(inputs["ln3_b"], f32)
    projw = np.asarray(inputs["proj_w"], f32)           # (V,H)
    projf = projw * g3[None, :]
    _B3VEC = (projw @ b3).astype(f32)                   # (V,) host-added bias

    shared = {
        "wqkvT": bfc(wqkv_packed.T),
        "woT": bfc(np.asarray(inputs["sa_out_w"], f32).T),
        "wcaT": bfc(wca),
        "w1T": bfc(w1f.T),
        "w2T": bfc(np.asarray(inputs["lin2_w"], f32).T),
        "bqkv": bfc(bqkv_packed),
        "bca": bfc(bca),
        "bres": bfc(bres),
        "b1": b1f.astype(f32),
        "g1": bfc(inputs["ln1_g"]),
        "g2": bfc(g2),
    }
    projT = np.zeros((H, VP), BF16)
    projT[:, :V] = projf.T.astype(BF16)
    shared["projT"] = projT

    bo = np.asarray(inputs["sa_out_b"], f32)
    tgt_bo = tgt + bo.reshape(1, 1, H)                  # residual + out bias

    in_maps = []
    for c in range(NCORES):
        sl = slice(c * S, (c + 1) * S)
        tgt_c = tgt[sl]                                       # (128,3,768)
        m = dict(shared)
        m["xT"] = np.ascontiguousarray(
            tgt_c.transpose(2, 1, 0).reshape(H, TOK)).astype(BF16)     # (768,384)
        m["xtok"] = np.ascontiguousarray(tgt_bo[sl]).astype(BF16)      # (128,3,768)
        m["memT"] = np.ascontiguousarray(mem[sl].T).astype(BF16)       # (768,128)
        in_maps.append(m)
    return in_maps


def kernel(**inputs):
    nc = _get_nc()
    in_maps = _prep_inputs(inputs)
    res = run_bass_kernel_spmd(nc, in_maps, core_ids=list(range(NCORES)))
    final = np.empty((B * T, NT, V), np.float32)
    for c in range(NCORES):
        final[c * S:(c + 1) * S] = res.results[c]["out"][:, :, :V].astype(np.float32)
    final += _B3VEC.reshape(1, 1, V)
    return final.reshape(B, T, NT, V)


# revision 16
# speedup vs baseline: 1.0353x; 1.0339x over previous
"""Trainium2 Bass kernel for nn_CausalMultiTokenPredictionHead.

Distribution: pure data parallel over the flattened B*T axis (1024 sequences
-> 128 per core x 8 cores). Each core runs the full 3-token causal decoder
layer for its 128 sequences and projects its 384 tokens against the full
(padded) vocab. Decoder weights + the vocab projection table are replicated.

Math notes (exact simplifications, no approximations beyond bf16 rounding):
  - Cross-attention has memory length 1 -> softmax over a single key is
    identically 1, so ca(x) = out_proj(v_proj(mem)) independent of x. The two
    projections collapse into ONE host-folded [H,H] matrix; the ln1 beta is
    folded into its bias.
  - Self-attention is over 3 tokens with a causal mask -> per-position
    closed-form softmax over <=3 scores on the vector engine. Scores are
    bounded (|s| < ~3) so the max-subtraction is skipped; 1/sqrt(hd) is
    folded into Wq on the host.
  - Position 0 attends only to itself, so o(p0) = v0; the kernel pushes
    position 0 through the decoder first so the vocab projection can start
    ~25us in and fill the tensor engine while positions 1/2 finish. The
    first few vocab weight groups are re-streamed later for p1/p2.
  - qkv is packed [v|k|q] so position 0 only computes its v columns.
  - LN2's affine folds into lin1_w/lin1_b (residual side applies g2 on DVE);
    LN3's affine folds into proj_w, with the beta3 @ proj_w^T vector added
    on the host after download (it is all-zeros in practice).
  - The tgt residual + sa_out bias are folded into a token-major upload of
    the target embeddings.
All matmuls run in bf16 (fp32 PSUM accumulation); layernorms, softmax and
the residual stream are fp32. All PE transposes run in bf16 (1 cycle/row).
Logits are written to DRAM as bf16 and upcast on the host.
"""
import numpy as np
import ml_dtypes

import concourse.bass as bass
import concourse.mybir as mybir
import concourse.tile as tile
from concourse import bacc
from concourse.bass_utils import run_bass_kernel_spmd
from concourse.masks import make_identity

BF16 = ml_dtypes.bfloat16
F32 = mybir.dt.float32
BF = mybir.dt.bfloat16

B, T, H, V, NT, NH, DFF = 2, 512, 768, 51865, 3, 4, 2048
EPS = 1e-5
NCORES = 8
S = 128                       # sequences per core
TOK = S * NT                  # tokens per core (pos-major: t = p*128 + s)
HT = H // 128                 # 6 h-tiles
FT = DFF // 128               # 16 dff-tiles
HD = H // NH                  # 192 head dim
VP = 52224                    # padded vocab (102 * 512)
VG = 1024                     # vocab columns per streamed weight group
NVG = VP // VG                # 51 groups
ACT = mybir.ActivationFunctionType
ALU = mybir.AluOpType

# packed qkv column chunks (<=512 for one PSUM bank); layout [v|k|q]
CH = [(0, 512), (512, 512), (1024, 512), (1536, 512), (2048, 256)]
CH0 = [(0, 512), (512, 256)]   # p0 only needs v (cols 0:768)
CH_H = [(0, 512), (512, 256)]  # 768 split

# projection job schedule: how many leading vocab groups run before
# x3T[p1] / x3T[p2] are ready (those groups are re-streamed later).
K0 = 3   # groups done p0-only
K1 = 5   # groups done p0+p1


def _bcast_load(nc, pool, dram, n, name, dtype=BF):
    """[n] DRAM vector -> [128, n] SBUF tile broadcast across partitions."""
    t = pool.tile([128, n], dtype, name=name, tag=name)
    ap = dram[:]
    bc = bass.AP(tensor=ap.tensor, offset=ap.offset, ap=[[0, 128]] + list(ap.ap))
    nc.gpsimd.dma_start(out=t[:], in_=bc)
    return t


def build_program():
    nc = bacc.Bacc(None, target_bir_lowering=False)

    # ---- DRAM I/O ----
    xT_d = nc.dram_tensor("xT", [H, TOK], BF, kind="ExternalInput")
    xtok_d = nc.dram_tensor("xtok", [S, NT, H], BF, kind="ExternalInput")
    memT_d = nc.dram_tensor("memT", [H, S], BF, kind="ExternalInput")
    wqkvT_d = nc.dram_tensor("wqkvT", [H, 3 * H], BF, kind="ExternalInput")
    woT_d = nc.dram_tensor("woT", [H, H], BF, kind="ExternalInput")
    wcaT_d = nc.dram_tensor("wcaT", [H, H], BF, kind="ExternalInput")
    w1T_d = nc.dram_tensor("w1T", [H, DFF], BF, kind="ExternalInput")
    w2T_d = nc.dram_tensor("w2T", [DFF, H], BF, kind="ExternalInput")
    projT_d = nc.dram_tensor("projT", [H, VP], BF, kind="ExternalInput")
    bqkv_d = nc.dram_tensor("bqkv", [3 * H], BF, kind="ExternalInput")
    bca_d = nc.dram_tensor("bca", [H], BF, kind="ExternalInput")
    bres_d = nc.dram_tensor("bres", [H], BF, kind="ExternalInput")
    b1_d = nc.dram_tensor("b1", [DFF], F32, kind="ExternalInput")
    g1_d = nc.dram_tensor("g1", [H], BF, kind="ExternalInput")
    g2_d = nc.dram_tensor("g2", [H], BF, kind="ExternalInput")
    out_d = nc.dram_tensor("out", [S, NT, VP], BF, kind="ExternalOutput")

    with tile.TileContext(nc) as tc:
        consts = tc.alloc_tile_pool(name="consts", bufs=1)
        longl = tc.alloc_tile_pool(name="longl", bufs=1)
        projp = tc.alloc_tile_pool(name="projp", bufs=3)
        stagep = tc.alloc_tile_pool(name="stagep", bufs=3)
        tmpp = tc.alloc_tile_pool(name="tmpp", bufs=1)
        wbig = tc.alloc_tile_pool(name="wbig", bufs=2)
        ffnp = tc.alloc_tile_pool(name="ffnp", bufs=1)
        psmm = tc.alloc_tile_pool(name="psmm", bufs=6, space="PSUM")
        pstp = tc.alloc_tile_pool(name="pstp", bufs=2, space="PSUM")

        # ---- constants ----
        ident_bf = consts.tile([128, 128], BF, name="ident_bf", tag="ident_bf")
        make_identity(nc, ident_bf)
        epst = consts.tile([128, 1], F32, name="epst", tag="epst")
        nc.vector.memset(epst, EPS)

        # ---- long-lived activations ----
        x3T = longl.tile([128, HT, TOK], BF, name="x3T", tag="x3T")
        x2T = longl.tile([128, HT, TOK], BF, name="x2T", tag="x2T")
        h1p_t = {}

        def scratch(name):
            return tmpp.tile([128, H], F32, name=name, tag="scratch", bufs=3)

        def ln_inplace(x_ap, name):
            """LayerNorm (no affine) along free dim (768) of [128,768] fp32."""
            stats = tmpp.tile([128, 3, 6], F32, name=f"st_{name}", tag="ln_stats", bufs=2)
            mv = tmpp.tile([128, 2], F32, name=f"mv_{name}", tag="ln_mv", bufs=4)
            xg = x_ap.rearrange("p (sg d) -> p sg d", sg=3)
            for sg in range(3):
                nc.vector.bn_stats(out=stats[:, sg, :], in_=xg[:, sg, :])
            nc.vector.bn_aggr(out=mv[:], in_=stats[:])
            nc.scalar.activation(out=mv[:, 1:2], in_=mv[:, 1:2], func=ACT.Sqrt,
                                 bias=epst[:], scale=1.0)
            nc.vector.reciprocal(out=mv[:, 1:2], in_=mv[:, 1:2])
            nc.vector.tensor_scalar(out=x_ap, in0=x_ap, scalar1=mv[:, 0:1],
                                    scalar2=mv[:, 1:2],
                                    op0=ALU.subtract, op1=ALU.mult)

        def transpose_bf(dst_ap, src_bf_ap):
            """[128,128] bf16 transpose through the PE (1 cycle/row)."""
            pt = pstp.tile([128, 128], BF, name="pt", tag="tp")
            nc.tensor.transpose(pt[:], src_bf_ap, ident_bf[:])
            nc.vector.tensor_copy(out=dst_ap, in_=pt[:])

        def cast_transpose(dstT, src_f32, p, name):
            """f32 [128,768] token-major -> bf16 h-major dstT[:, :, p*128:...]."""
            xb = tmpp.tile([128, H], BF, name=f"cb_{name}", tag="castb", bufs=3)
            nc.scalar.copy(out=xb[:], in_=src_f32)
            for hh in range(HT):
                transpose_bf(dstT[:, hh, p * 128:(p + 1) * 128],
                             xb[:, hh * 128:(hh + 1) * 128])

        # ---- big-weight rotation: wqkv -> w1 -> w2 share 2 slots ----
        wqkv_sb = wbig.tile([128, HT, 3 * H], BF, name="wqkv_sb", tag="wbig")
        w1_sb = wbig.tile([128, HT, DFF], BF, name="w1_sb", tag="wbig")
        w2_sb = wbig.tile([128, FT, H], BF, name="w2_sb", tag="wbig")

        decA = tc.alloc_tile_pool(name="decA", bufs=1)

        def sync_bcast(dram, n, name, dtype=BF):
            """broadcast load positioned on the (ordered) sync queue."""
            t = consts.tile([128, n], dtype, name=name, tag=name)
            ap = dram[:]
            bc = bass.AP(tensor=ap.tensor, offset=ap.offset, ap=[[0, 128]] + list(ap.ap))
            nc.sync.dma_start(out=t[:], in_=bc)
            return t

        # ============ DMA issue: ONE queue (sync), strict priority order ====
        # need-times: xT/v-chunks (qkv p0) < wo/xtok (sa p0) < memT/wca (CA)
        # < w1 (lin1 p0) < k/q chunks (attn p1/p2) < w2 < proj groups
        xT_sb = decA.tile([128, HT, TOK], BF, name="xT_sb", tag="xT_sb")
        nc.sync.dma_start(out=xT_sb[:], in_=xT_d[:].rearrange("(ht p) t -> p ht t", p=128))
        wqkv_r = wqkvT_d[:].rearrange("(ht p) o -> p ht o", p=128)
        for (c0, cn) in CH[:2]:
            nc.sync.dma_start(out=wqkv_sb[:, :, c0:c0 + cn], in_=wqkv_r[:, :, c0:c0 + cn])
        bqkv_bc = sync_bcast(bqkv_d, 3 * H, "bqkv_bc")
        wo_sb = decA.tile([128, HT, H], BF, name="wo_sb", tag="w_med", bufs=2)
        nc.sync.dma_start(out=wo_sb[:], in_=woT_d[:].rearrange("(ht p) o -> p ht o", p=128))
        xtok_sb = decA.tile([128, NT, H], BF, name="xtok_sb", tag="xtok_sb")
        nc.sync.dma_start(out=xtok_sb[:], in_=xtok_d[:])
        memT_sb = decA.tile([128, HT, S], BF, name="memT_sb", tag="memT_sb")
        nc.sync.dma_start(out=memT_sb[:], in_=memT_d[:].rearrange("(ht p) s -> p ht s", p=128))
        wca_sb = decA.tile([128, HT, H], BF, name="wca_sb", tag="w_med", bufs=2)
        nc.sync.dma_start(out=wca_sb[:], in_=wcaT_d[:].rearrange("(ht p) o -> p ht o", p=128))
        bca_bc = sync_bcast(bca_d, H, "bca_bc")
        g1_bc = sync_bcast(g1_d, H, "g1_bc")
        w1_r = w1T_d[:].rearrange("(ht p) o -> p ht o", p=128)
        for c0 in range(0, DFF, 512):
            nc.sync.dma_start(out=w1_sb[:, :, c0:c0 + 512], in_=w1_r[:, :, c0:c0 + 512])
        b1_sb = consts.tile([128, FT], F32, name="b1_sb", tag="b1_sb")
        nc.sync.dma_start(out=b1_sb[:], in_=b1_d[:].rearrange("(ft p) -> p ft", p=128))
        for (c0, cn) in CH[2:]:
            nc.sync.dma_start(out=wqkv_sb[:, :, c0:c0 + cn], in_=wqkv_r[:, :, c0:c0 + cn])
        g2_bc = sync_bcast(g2_d, H, "g2_bc")
        bres_bc = sync_bcast(bres_d, H, "bres_bc")
        w2_r = w2T_d[:].rearrange("(ft p) o -> p ft o", p=128)
        for f0 in range(0, FT, 4):
            nc.sync.dma_start(out=w2_sb[:, f0:f0 + 4, :], in_=w2_r[:, f0:f0 + 4, :])

        # ---- PE warm-up / gap filler: dummy transposes keep the p-state
        # ramp hot while PE waits on DMA (it would idle and cold-reset).
        def dummies(k):
            for _ in range(k):
                pt = pstp.tile([128, 128], BF, name="pt", tag="tp")
                nc.tensor.transpose(pt[:], ident_bf[:], ident_bf[:])

        # ================= decoder compute =================
        qkv = decA.tile([128, NT, 3 * H], BF, name="qkv", tag="qkv")

        def qkv_chunk(ci, plist):
            c0, cn = CH[ci]
            for p in plist:
                if p == 0:
                    if c0 >= 768:
                        continue
                    cn_p = min(cn, 768 - c0)
                else:
                    cn_p = cn
                ps = psmm.tile([128, 512], F32, name="ps_qkv", tag="mm")[:, :cn_p]
                for h in range(HT):
                    nc.tensor.matmul(ps, xT_sb[:, h, p * 128:(p + 1) * 128],
                                     wqkv_sb[:, h, c0:c0 + cn_p],
                                     start=(h == 0), stop=(h == HT - 1))
                nc.vector.tensor_tensor(qkv[:, p, c0:c0 + cn_p], ps,
                                        bqkv_bc[:, c0:c0 + cn_p], ALU.add)

        def cross_attention():
            # ONE folded matmul; epilogue adds (bias + ln1_b) -> cab
            for (c0, cn) in CH_H:
                ps = psmm.tile([128, 512], F32, name="ps_ca", tag="mm")[:, :cn]
                for h in range(HT):
                    nc.tensor.matmul(ps, memT_sb[:, h, :], wca_sb[:, h, c0:c0 + cn],
                                     start=(h == 0), stop=(h == HT - 1))
                nc.vector.tensor_tensor(cab[:, c0:c0 + cn], ps,
                                        bca_bc[:, c0:c0 + cn], ALU.add)

        cab = decA.tile([128, H], F32, name="cab", tag="cab")

        # --- per-position SA out-proj + residual + LN1 + (+cab) + LN2 ---
        oT = decA.tile([128, HT, TOK], BF, name="oT", tag="oT")
        x2h_t = {}

        def sa_x1(p):
            """sa out-proj matmuls + the DVE chain through LN2 (no PE tail)."""
            x1p = tmpp.tile([128, H], F32, name=f"x1_{p}", tag="x1p", bufs=3)
            for (c0, cn) in CH_H:
                ps = psmm.tile([128, 512], F32, name="ps_sao", tag="mm")[:, :cn]
                for h in range(HT):
                    nc.tensor.matmul(ps, oT[:, h, p * 128:(p + 1) * 128],
                                     wo_sb[:, h, c0:c0 + cn],
                                     start=(h == 0), stop=(h == HT - 1))
                # residual (tgt + bo) folded into xtok upload
                nc.vector.tensor_tensor(x1p[:, c0:c0 + cn], ps,
                                        xtok_sb[:, p, c0:c0 + cn], ALU.add)
            ln_inplace(x1p[:], f"ln1_{p}")
            nc.vector.tensor_tensor(x1p[:], x1p[:], g1_bc[:, :], ALU.mult)
            nc.vector.tensor_tensor(x1p[:], x1p[:], cab[:], ALU.add)
            ln_inplace(x1p[:], f"ln2_{p}")
            x2h_t[p] = x1p

        def x2_finish(p):
            cast_transpose(x2T, x2h_t[p][:], p, f"x2_{p}")

        # --- FFN pieces ---
        def lin1_p(p):
            h1p = ffnp.tile([128, FT, 128], BF, name=f"h1_{p}", tag="h1p", bufs=2)
            h1p_t[p] = h1p
            t0 = p * 128
            for ft in range(FT):
                ps = psmm.tile([128, 512], F32, name="ps_l1", tag="mm")[:, :128]
                for h in range(HT):
                    nc.tensor.matmul(ps, w1_sb[:, h, ft * 128:(ft + 1) * 128],
                                     x2T[:, h, t0:t0 + 128],
                                     start=(h == 0), stop=(h == HT - 1))
                nc.scalar.activation(out=h1p[:, ft, :], in_=ps, func=ACT.Relu,
                                     bias=b1_sb[:, ft:ft + 1], scale=1.0)
                if p == 0 and ft % 4 == 3 and ft < 12:
                    dummies(6)

        def ffn_tail(p):
            # residual side: x2g = ln2hat * g2 + (lin2_b + ln2_b)   (gpsimd)
            x2g = x2h_t[p]
            nc.gpsimd.tensor_tensor(x2g[:], x2g[:], g2_bc[:, :], ALU.mult)
            nc.gpsimd.tensor_tensor(x2g[:], x2g[:], bres_bc[:, :], ALU.add)
            x3p = scratch(f"x3_{p}")
            # both 512/256 PSUM chains advance together per ft so the chain
            # finishes right as the last streamed w2 chunk lands (p0 path)
            ps_a = psmm.tile([128, 512], F32, name="ps_l2a", tag="mm")
            ps_b = psmm.tile([128, 512], F32, name="ps_l2b", tag="mm")[:, :256]
            for ft in range(FT):
                nc.tensor.matmul(ps_a, h1p_t[p][:, ft, :], w2_sb[:, ft, 0:512],
                                 start=(ft == 0), stop=(ft == FT - 1))
                nc.tensor.matmul(ps_b, h1p_t[p][:, ft, :], w2_sb[:, ft, 512:768],
                                 start=(ft == 0), stop=(ft == FT - 1))
                if p == 0 and ft % 4 == 3 and ft < 12:
                    dummies(8)
            nc.vector.tensor_tensor(x3p[:, 0:512], ps_a, x2g[:, 0:512], ALU.add)
            nc.vector.tensor_tensor(x3p[:, 512:768], ps_b, x2g[:, 512:768], ALU.add)
            ln_inplace(x3p[:], f"ln3_{p}")
            cast_transpose(x3T, x3p[:], p, f"x3_{p}")

        # ===== attention math for positions 1,2 (DVE; scores pre-scaled) =====
        w_t = {}

        def vheads(j):
            return qkv[:, j, 0:H].rearrange("p (nh hd) -> p nh hd", nh=NH)

        def wb(i, j):
            return w_t[i][:, j, :, None].to_broadcast((128, NH, HD))

        def attn(i):
            # the [768]-wide ops for position 2 run on the (idle) Pool engine
            # so the p1 and p2 chains overlap; tiny softmax ops stay on DVE.
            eng = nc.gpsimd if i == 2 else nc.vector
            nj = i + 1
            s = decA.tile([128, 3, NH], F32, name=f"s{i}", tag=f"s{i}")[:, :nj, :]
            for j in range(nj):
                prod = scratch(f"prod{i}{j}")
                eng.tensor_tensor(prod[:], qkv[:, i, 2 * H:3 * H],
                                  qkv[:, j, H:2 * H], ALU.mult)
                nc.vector.reduce_sum(out=s[:, j, :],
                                     in_=prod[:].rearrange("p (nh hd) -> p nh hd", nh=NH),
                                     axis=mybir.AxisListType.X)
            e = tmpp.tile([128, 3, NH], F32, name=f"e{i}", tag="sm_e", bufs=2)[:, :nj, :]
            nc.scalar.activation(out=e, in_=s, func=ACT.Exp)
            den = tmpp.tile([128, NH], F32, name=f"den{i}", tag="sm_small", bufs=8)
            nc.vector.reduce_sum(out=den[:], in_=e.rearrange("p j h -> p h j"),
                                 axis=mybir.AxisListType.X)
            nc.vector.reciprocal(out=den[:], in_=den[:])
            w = decA.tile([128, 3, NH], F32, name=f"w{i}", tag=f"w{i}")[:, :nj, :]
            nc.vector.tensor_tensor(w, e, den[:, None, :].to_broadcast((128, nj, NH)),
                                    ALU.mult)
            w_t[i] = w
            facc = scratch(f"facc{i}")
            tmp3 = scratch(f"tmp3{i}")
            fv = facc[:].rearrange("p (nh hd) -> p nh hd", nh=NH)
            tv = tmp3[:].rearrange("p (nh hd) -> p nh hd", nh=NH)
            eng.tensor_tensor(fv, vheads(0), wb(i, 0), ALU.mult)
            eng.tensor_tensor(tv, vheads(1), wb(i, 1), ALU.mult)
            if i == 1:
                o_i = scratch("o1")
                eng.tensor_tensor(o_i[:], facc[:], tmp3[:], ALU.add)
            else:
                eng.tensor_tensor(facc[:], facc[:], tmp3[:], ALU.add)
                eng.tensor_tensor(tv, vheads(2), wb(2, 2), ALU.mult)
                o_i = scratch("o2")
                eng.tensor_tensor(o_i[:], facc[:], tmp3[:], ALU.add)
            # o_i f32 -> bf16 (Act) -> PE transpose into oT
            ob = tmpp.tile([128, H], BF, name=f"ob{i}", tag="castb", bufs=3)
            nc.scalar.copy(out=ob[:], in_=o_i[:])
            for hh in range(HT):
                transpose_bf(oT[:, hh, i * 128:(i + 1) * 128],
                             ob[:, hh * 128:(hh + 1) * 128])

        # ================= vocab projection machinery =================
        projT_r = projT_d[:].rearrange("(ht p) v -> p ht v", p=128)
        wt_ring = {}

        def proj_load(job_idx, vg):
            wt = projp.tile([128, HT, VG], BF, name="wt", tag="projw")
            nc.sync.dma_start(out=wt[:], in_=projT_r[:, :, vg * VG:(vg + 1) * VG])
            wt_ring[job_idx] = wt

        pending_stores = []

        def flush_stores():
            for (dst, src) in pending_stores:
                nc.scalar.dma_start(out=dst, in_=src)
            pending_stores.clear()

        def proj_compute(job_idx, vg, plist, tail=False, act_only=False):
            wt = wt_ring.pop(job_idx)
            for p in plist:
                stg = stagep.tile([128, VG], BF, name=f"stg{p}", tag="stg", bufs=6)
                nhalf = VG // 512
                for half in range(nhalf):
                    hw_ = 160 if (vg == NVG - 1 and half == 1) else 512
                    ps = psmm.tile([128, 512], F32, name="ps_pr", tag="mm")[:, :hw_]
                    for h in range(HT):
                        nc.tensor.matmul(ps, x3T[:, h, p * 128:(p + 1) * 128],
                                         wt[:, h, half * 512:half * 512 + hw_],
                                         start=(h == 0), stop=(h == HT - 1))
                    dst = stg[:, half * 512:half * 512 + hw_]
                    # act_only: keep DVE free for the decoder's LN chains
                    if half % 2 == 0 and not act_only:
                        nc.vector.tensor_copy(out=dst, in_=ps)
                    else:
                        nc.scalar.copy(out=dst, in_=ps)
                    if tail:  # flush per-half to shrink the exit tail
                        nc.scalar.dma_start(
                            out=out_d[:, p, vg * VG + half * 512: vg * VG + half * 512 + hw_],
                            in_=stg[:, half * 512:half * 512 + hw_])
                if not tail:
                    # deferred: emitted at the next job so the store's wait
                    # never blocks the Act sequencer behind unmet deps
                    pending_stores.append((out_d[:, p, vg * VG:(vg + 1) * VG],
                                           stg[:]))

        # ================= emission schedule =================
        # PE order == emission order (in-order engine). p0 fast path first;
        # dummies() fills known DMA-gated stalls to hold the p-state ramp.
        dummies(26)
        qkv_chunk(0, [0, 1, 2])
        qkv_chunk(1, [0, 1, 2])
        dummies(8)
        for hh in range(HT):   # o(p0) = v0 (already bf16 in qkv)
            transpose_bf(oT[:, hh, 0:128],
                         qkv[:, 0, hh * 128:(hh + 1) * 128])
        sa_x1(0)
        dummies(20)
        cross_attention()
        dummies(26)
        x2_finish(0)
        lin1_p(0)
        qkv_chunk(2, [1, 2])
        qkv_chunk(3, [1, 2])
        qkv_chunk(4, [1, 2])
        attn(1)                # DVE chain for p1
        attn(2)                # Pool chain for p2
        ffn_tail(0)            # w2-streamed; -> x3T p0 ready

        # projection jobs interleaved with the p1/p2 decoder tail.
        # job list: (vg, plist) in emission order; re-streams at the end.
        jobs = []
        for vg in range(K0):
            jobs.append((vg, [0]))
        for vg in range(K0, K1):
            jobs.append((vg, [0, 1]))
        for vg in range(K1, NVG):
            jobs.append((vg, [0, 1, 2]))
        for vg in range(K0):
            jobs.append((vg, [1, 2]))
        for vg in range(K0, K1):
            jobs.append((vg, [2]))

        # decoder-tail emission points: before job n, emit these steps
        tail_steps = {
            0: [lambda: sa_x1(1)],
            1: [lambda: x2_finish(1), lambda: lin1_p(1)],
            2: [lambda: ffn_tail(1), lambda: sa_x1(2)],
            3: [lambda: x2_finish(2), lambda: lin1_p(2)],
            4: [lambda: ffn_tail(2)],
        }

        proj_load(0, jobs[0][0])
        proj_load(1, jobs[1][0])
        for n, (vg, plist) in enumerate(jobs):
            if n + 2 < len(jobs):
                proj_load(n + 2, jobs[n + 2][0])
            for step in tail_steps.get(n, ()):
                step()
            flush_stores()
            proj_compute(n, vg, plist, tail=(n >= len(jobs) - 2),
                         act_only=(n < 6))
        flush_stores()

        decA.release()
        pstp.release()
        psmm.release()
        ffnp.release()
        wbig.release()
        tmpp.release()
        stagep.release()
        projp.release()
        longl.release()
        consts.release()

    nc.finalize()
    return nc


_NC_CACHE = None


def _get_nc():
    global _NC_CACHE
    if _NC_CACHE is None:
        _NC_CACHE = build_program()
    return _NC_CACHE


_B3VEC = None


def _prep_inputs(inputs):
    global _B3VEC
    f32 = np.float32
    enc = np.asarray(inputs["encoder_hidden"], f32)           # (B,T,H)
    tok = np.asarray(inputs["teacher_tokens"]).astype(np.int64)
    emb = np.asarray(inputs["emb"], f32)
    start = np.asarray(inputs["start_token"], f32)
    N = B * T

    tgt = np.empty((N, NT, H), f32)
    tgt[:, 0, :] = start.reshape(1, H)
    tgt[:, 1:, :] = emb[tok.reshape(N, NT)[:, : NT - 1]]
    mem = enc.reshape(N, H)

    def bfc(a):
        return np.ascontiguousarray(np.asarray(a, dtype=f32)).astype(BF16)

    # ---- host-side weight algebra (folds) ----
    wq = np.asarray(inputs["sa_in_w"], f32)[0:H]        # (H,H): q = x @ wq.T
    wk = np.asarray(inputs["sa_in_w"], f32)[H:2 * H]
    wv = np.asarray(inputs["sa_in_w"], f32)[2 * H:]
    bq = np.asarray(inputs["sa_in_b"], f32)[0:H]
    bk = np.asarray(inputs["sa_in_b"], f32)[H:2 * H]
    bv = np.asarray(inputs["sa_in_b"], f32)[2 * H:]
    c_inv = 1.0 / np.sqrt(np.float32(HD))
    # packed [v|k|q], 1/sqrt(hd) folded into q
    wqkv_packed = np.concatenate([wv, wk, wq * c_inv], axis=0)       # (3H, H)
    bqkv_packed = np.concatenate([bv, bk, bq * c_inv], axis=0)

    ca_wv = np.asarray(inputs["ca_in_w"], f32)[2 * H:]  # (H,H)
    ca_bv = np.asarray(inputs["ca_in_b"], f32)[2 * H:]
    ca_wo = np.asarray(inputs["ca_out_w"], f32)         # (H,H)
    ca_bo = np.asarray(inputs["ca_out_b"], f32)
    # ca(mem) = mem @ (ca_wv.T @ ca_wo.T) + (ca_bv @ ca_wo.T + ca_bo)
    wca = ca_wv.T @ ca_wo.T                             # (H,H) input-major
    bca = ca_bv @ ca_wo.T + ca_bo + np.asarray(inputs["ln1_b"], f32)

    g2 = np.asarray(inputs["ln2_g"], f32)
    b2 = np.asarray(inputs["ln2_b"], f32)
    w1 = np.asarray(inputs["lin1_w"], f32)              # (DFF,H)
    w1f = w1 * g2[None, :]
    b1f = np.asarray(inputs["lin1_b"], f32) + w1 @ b2
    bres = np.asarray(inputs["lin2_b"], f32) + b2

    g3 = np.asarray(inputs["ln3_g"], f32)
    b3 = np.asarray(inputs["ln3_b"], f32)
    projw = np.asarray(inputs["proj_w"], f32)           # (V,H)
    projf = projw * g3[None, :]
    _B3VEC = (projw @ b3).astype(f32)                   # (V,) host-added bias

    shared = {
        "wqkvT": bfc(wqkv_packed.T),
        "woT": bfc(np.asarray(inputs["sa_out_w"], f32).T),
        "wcaT": bfc(wca),
        "w1T": bfc(w1f.T),
        "w2T": bfc(np.asarray(inputs["lin2_w"], f32).T),
        "bqkv": bfc(bqkv_packed),
        "bca": bfc(bca),
        "bres": bfc(bres),
        "b1": b1f.astype(f32),
        "g1": bfc(inputs["ln1_g"]),
        "g2": bfc(g2),
    }
    projT = np.zeros((H, VP), BF16)
    projT[:, :V] = projf.T.astype(BF16)
    shared["projT"] = projT

    bo = np.asarray(inputs["sa_out_b"], f32)
    tgt_bo = tgt + bo.reshape(1, 1, H)                  # residual + out bias

    in_maps = []
    for c in range(NCORES):
        sl = slice(c * S, (c + 1) * S)
        tgt_c = tgt[sl]                                       # (128,3,768)
        m = dict(shared)
        m["xT"] = np.ascontiguousarray(
            tgt_c.transpose(2, 1, 0).reshape(H, TOK)).astype(BF16)     # (768,384)
        m["xtok"] = np.ascontiguousarray(tgt_bo[sl]).astype(BF16)      # (128,3,768)
        m["memT"] = np.ascontiguousarray(mem[sl].T).astype(BF16)       # (768,128)
        in_maps.append(m)
    return in_maps


def kernel(**inputs):
    nc = _get_nc()
    in_maps = _prep_inputs(inputs)
    res = run_bass_kernel_spmd(nc, in_maps, core_ids=list(range(NCORES)))
    final = np.empty((B * T, NT, V), np.float32)
    for c in range(NCORES):
        final[c * S:(c + 1) * S] = res.results[c]["out"][:, :, :V].astype(np.float32)
    final += _B3VEC.reshape(1, 1, V)
    return final.reshape(B, T, NT, V)
